# revision 3
# baseline (speedup 1.0000x reference)
"""Trainium2 Bass kernel for nn_DenSparseMatrix (gnn_message_passing).

Math: out[b, o] = sum_k rm[o,k] * s[idx[o,k], k] * x[b, idx[o,k]],
      s = forward_weights * forward_mask  (elementwise, [I, W])

Strategy (8 NeuronCores, SPMD):
  * rm and fm are 0/1-valued; only ~1/4 of the (o, k) tokens have a
    nonzero coefficient c[o,k] = rm[o,k] * s[idx[o,k], k].  The host
    computes c, drops zero tokens, and sorts outputs by surviving token
    count so each 128-output block has a near-uniform count T (padding
    to the block max costs <1%).  Sorted blocks are dealt round-robin to
    the 8 cores; block position bi uses T_list[bi] = max over the 8
    dealt blocks, so all cores share one SPMD program.
  * The gather table is x^T in bf16 pairs padded to the 256B descriptor
    minimum: row j = [x[:,2j] | x[:,2j+1] | 64B pad] (128 bf16), 32768
    rows so the 15-bit int16 gather index j = idx >> 1 reaches all
    65536 inputs.  Shipped as an ExternalInput already in table layout.
  * Gathers are merged GS=4 blocks per gpsimd.dma_gather to amortize
    the ~1us SWDGE fixed overhead (desc-gen on the Pool engine is the
    serial bottleneck: one Q7 core pair per queue, ~1.5ns/descriptor).
    Gathers rotate over the 4 SWDGE queues.
  * One DVE multiply per group applies the parity-split bf16
    coefficients; one reduce per block sums (slot, parity) into f32.
"""

import numpy as np

import concourse.bass as bass
import concourse.bacc as bacc
import concourse.mybir as mybir
from concourse.tile import TileContext
from concourse.bass_utils import run_bass_kernel_spmd
from concourse.library_config import mlp

I = 65536
O = 65536
W = 32
B = 32
NCORES = 8
NBLK = (O // NCORES) // 128   # 64 block positions per core
GS = 4                        # blocks merged per gather
NQ = 4                        # SWDGE queues used round-robin
F32 = mybir.dt.float32
BF16 = mybir.dt.bfloat16
I16 = mybir.dt.int16
NPBF16 = mybir.dt.np(BF16)
ROW = 4 * B                   # 128 bf16 = 256B table row


def _build_nc(t_list):
    sum_t = sum(t_list)
    groups = [t_list[g:g + GS] for g in range(0, NBLK, GS)]
    tg_max = max(sum(g) for g in groups)
    nc = bacc.Bacc("TRN2", target_bir_lowering=False, debug=False,
                   num_devices=NCORES, num_swdge_queues=NQ)

    tab_d = nc.dram_tensor("tab", [I // 2, ROW], BF16, kind="ExternalInput")
    idx_d = nc.dram_tensor("idx", [128, 8 * sum_t], I16, kind="ExternalInput")
    c01_d = nc.dram_tensor("c01", [128, 2 * sum_t], BF16, kind="ExternalInput")
    out_d = nc.dram_tensor("out", [128, NBLK * B], F32, kind="ExternalOutput")

    with TileContext(nc) as tc:
        nc.gpsimd.load_library(mlp)

        with (
            tc.tile_pool(name="pres", bufs=1) as pres,
            tc.tile_pool(name="pg", bufs=4) as pg,
            tc.tile_pool(name="ptmp", bufs=3) as ptmp,
        ):
            idx_all = pres.tile([128, 8 * sum_t], I16)
            nc.sync.dma_start(idx_all[:], idx_d[:])
            c01_all = pres.tile([128, 2 * sum_t], BF16)
            nc.sync.dma_start(c01_all[:], c01_d[:])
            ocore = pres.tile([128, NBLK * B], F32)

            off = 0
            for g, gts in enumerate(groups):
                tg = sum(gts)
                if tg == 0:
                    for bj in range(GS):
                        nc.vector.memset(
                            ocore[:, (g * GS + bj) * B:(g * GS + bj + 1) * B],
                            0.0)
                    continue
                G = pg.tile([128, tg_max, ROW], BF16, tag="G")
                nc.gpsimd.dma_gather(
                    G[:, :tg, :], tab_d[:, :],
                    idx_all[:, 8 * off:8 * (off + tg)],
                    128 * tg, 128 * tg, ROW,
                    single_packet=False, queue_num=g % NQ)

                gv = G[:]
                cv = c01_all[:, 2 * off:2 * (off + tg)]
                tmp = ptmp.tile([128, B, 2 * tg_max], BF16, tag="tmp")
                tv = tmp[:]
                # tmp[p, b, 2t+h] = G[p, t, B*h + b] * c01[p, 2t+h]
                gx = bass.AP(gv.tensor, gv.offset,
                             [list(gv.ap[0]), [ROW, tg], [B, 2], [1, B]])
                ab = bass.AP(cv.tensor, cv.offset,
                             [list(cv.ap[0]), [2, tg], [1, 2], [0, B]])
                t_ap = bass.AP(tv.tensor, tv.offset,
                               [list(tv.ap[0]), [2, tg], [1, 2],
                                [2 * tg_max, B]])
                nc.vector.tensor_mul(t_ap, gx, ab)

                goff = 0
                for bj, T in enumerate(gts):
                    osl = ocore[:, (g * GS + bj) * B:(g * GS + bj + 1) * B]
                    if T == 0:
                        nc.vector.memset(osl, 0.0)
                        continue
                    red_in = bass.AP(tv.tensor, tv.offset + 2 * goff,
                                     [list(tv.ap[0]), [2 * tg_max, B],
                                      [1, 2 * T]])
                    nc.vector.reduce_sum(osl, red_in,
                                         axis=mybir.AxisListType.X)
                    goff += T
                off += tg

            nc.sync.dma_start(out_d[:], ocore[:])

    nc.compile()
    return nc


def make_plan(x, forward_weights, forward_mask, output_mapping, reverse_mask):
    """Host-side analysis: nonzero-coefficient tokens, sorted block layout."""
    idx = np.asarray(output_mapping).astype(np.int64)
    rm = np.asarray(reverse_mask, dtype=np.float32)
    s = (np.asarray(forward_weights, dtype=np.float32)
         * np.asarray(forward_mask, dtype=np.float32))
    cols = np.arange(W)[None, :]
    c = rm * s[idx, cols]                                  # [O, W]
    nz = c != 0
    cnt = nz.sum(1)
    nzorder = np.argsort(~nz, axis=1, kind="stable")       # nonzero k's first
    order = np.argsort(-cnt, kind="stable")                # outputs by count desc
    bmax = cnt[order].reshape(O // 128, 128).max(1)        # per sorted block
    t_list = tuple(int(bmax[8 * bi:8 * bi + 8].max()) for bi in range(NBLK))
    return {"idx": idx, "c": c, "nzorder": nzorder, "order": order,
            "t_list": t_list}


def make_in_maps(x, plan):
    x = np.asarray(x, dtype=np.float32)
    xpair = x.T.astype(NPBF16).reshape(I // 2, 2 * B)
    tab = np.zeros((I // 2, ROW), NPBF16)
    tab[:, :2 * B] = xpair
    idx, c, nzorder, order, t_list = (
        plan["idx"], plan["c"], plan["nzorder"], plan["order"], plan["t_list"])

    in_maps = []
    for core in range(NCORES):
        idx_parts, c_parts = [], []
        for bi, T in enumerate(t_list):
            g = (8 * bi + core) * 128
            outs = order[g:g + 128]                        # [128]
            if T == 0:
                continue
            sel = nzorder[outs, :T]                        # [128, T]
            ii = idx[outs[:, None], sel]                   # [128, T]
            J = (ii >> 1).astype(np.int16)
            H = (ii & 1).astype(np.int64)
            CV = c[outs[:, None], sel].astype(np.float32)  # zero past cnt
            # token m = s*128 + p; wrap in 16 partitions, replicate x8
            L = J.T.reshape(8 * T, 16).T                   # [16, 8T]
            idx_parts.append(np.tile(L, (8, 1)))           # [128, 8T]
            c01 = np.zeros((128, T, 2), np.float32)
            np.put_along_axis(c01, H[:, :, None], CV[:, :, None], axis=2)
            c_parts.append(c01.reshape(128, 2 * T))
        in_maps.append({
            "tab": tab,
            "idx": np.ascontiguousarray(np.concatenate(idx_parts, axis=1)),
            "c01": np.ascontiguousarray(
                np.concatenate(c_parts, axis=1)).astype(NPBF16),
        })
    return in_maps


def unshard_out(results, plan):
    order = plan["order"]
    out = np.empty((B, O), np.float32)
    for core in range(NCORES):
        oc = results[core]["out"]                          # [128, NBLK*B]
        vals = oc.reshape(128, NBLK, B).transpose(2, 1, 0) # [B, NBLK, 128]
        perm = order.reshape(NBLK, NCORES, 128)[:, core, :].reshape(-1)
        out[:, perm] = vals.reshape(B, NBLK * 128)
    return out


_NC = None
_NC_KEY = None


def _get_nc(t_list):
    global _NC, _NC_KEY
    if _NC is None or _NC_KEY != t_list:
        _NC = _build_nc(t_list)
        _NC_KEY = t_list
    return _NC


def kernel(x, forward_weights, forward_mask, output_mapping, reverse_mask):
    plan = make_plan(x, forward_weights, forward_mask,
                     output_mapping, reverse_mask)
    nc = _get_nc(plan["t_list"])
    in_maps = make_in_maps(x, plan)
    res = run_bass_kernel_spmd(nc, in_maps, core_ids=list(range(NCORES)))
    return unshard_out(res.results, plan)


# revision 4
# speedup vs baseline: 2.2363x; 2.2363x over previous
"""Trainium2 Bass kernel for nn_DenSparseMatrix (gnn_message_passing).

Math: out[b, o] = sum_k rm[o,k] * s[idx[o,k], k] * x[b, idx[o,k]],
      s = forward_weights * forward_mask  (elementwise, [I, W])

Strategy (8 NeuronCores, SPMD).  SWDGE descriptor generation on the Pool
engine (one Q7 core pair, ~1.5ns/descriptor + ~1us/instruction) is the
serial bottleneck, so the host works to minimize descriptor count:

  * rm and fm are 0/1-valued; only ~1/4 of the (o, k) tokens have a
    nonzero coefficient c[o,k] = rm[o,k] * s[idx[o,k], k].  Zero tokens
    are dropped host-side.
  * Outputs are assigned to cores by greedy graph coloring so outputs
    sharing an input land on different cores (an input used twice on
    one core forces two separate row fetches).
  * Each core gets its own gather table: a permutation of the x columns
    into 256B pair rows [x[:,ia] | x[:,ib]], paired so that both halves
    of a fetched row usually belong to the SAME output (one descriptor
    then covers two tokens).  Tokens whose input is already placed
    reuse that row.  ~42k descriptors/core vs 262k for the dense case.
  * Within each core outputs are sorted by row count so each 128-output
    block has near-uniform T; block position bi uses the max T over the
    8 cores, so all cores share one SPMD program.
  * Gathers are merged up to a 2048-descriptor cap per instruction
    (larger instructions stall on ring backpressure), rotating over the
    4 SWDGE queues.  DVE applies the per-half coefficients and reduces.
"""

import numpy as np

import concourse.bass as bass
import concourse.bacc as bacc
import concourse.mybir as mybir
from concourse.tile import TileContext
from concourse.bass_utils import run_bass_kernel_spmd
from concourse.library_config import mlp

I = 65536
O = 65536
W = 32
B = 32
NCORES = 8
NROWS = I // 2                # 32768 table rows (int16 gather index limit)
NBLK = (O // NCORES) // 128   # 64 block positions per core
DESC_CAP = 2048               # max descriptors per merged gather
NQ = 4                        # SWDGE queues used round-robin
F32 = mybir.dt.float32
I16 = mybir.dt.int16


def _build_nc(t_list, groups):
    sum_t = sum(t_list)
    t_max = max(t_list)
    cap_t = max(sum(t_list[a:b]) for a, b in groups)
    nc = bacc.Bacc("TRN2", target_bir_lowering=False, debug=False,
                   num_devices=NCORES, num_swdge_queues=NQ)

    tab_d = nc.dram_tensor("tab", [NROWS, 2 * B], F32, kind="ExternalInput")
    idx_d = nc.dram_tensor("idx", [128, 8 * sum_t], I16, kind="ExternalInput")
    c01_d = nc.dram_tensor("c01", [128, 2 * sum_t], F32, kind="ExternalInput")
    out_d = nc.dram_tensor("out", [128, NBLK * B], F32, kind="ExternalOutput")

    with TileContext(nc) as tc:
        nc.gpsimd.load_library(mlp)

        with (
            tc.tile_pool(name="pres", bufs=1) as pres,
            tc.tile_pool(name="pg", bufs=6) as pg,
            tc.tile_pool(name="ptmp", bufs=3) as ptmp,
        ):
            idx_all = pres.tile([128, 8 * sum_t], I16)
            nc.sync.dma_start(idx_all[:], idx_d[:])
            c01_all = pres.tile([128, 2 * sum_t], F32)
            nc.sync.dma_start(c01_all[:], c01_d[:])
            ocore = pres.tile([128, NBLK * B], F32)

            goff = [0]
            for a, b in groups:
                goff.append(goff[-1] + sum(t_list[a:b]))

            for g, (a, bnd) in enumerate(groups):
                tg = sum(t_list[a:bnd])
                off = goff[g]
                if tg == 0:
                    for bi in range(a, bnd):
                        nc.vector.memset(
                            ocore[:, bi * B:(bi + 1) * B], 0.0)
                    continue
                G = pg.tile([128, cap_t, 2 * B], F32, tag="G")
                nc.gpsimd.dma_gather(
                    G[:, :tg, :], tab_d[:, :],
                    idx_all[:, 8 * off:8 * (off + tg)],
                    128 * tg, 128 * tg, 2 * B,
                    single_packet=False, queue_num=g % NQ)

                gv = G[:]
                boff = 0
                for bi in range(a, bnd):
                    T = t_list[bi]
                    osl = ocore[:, bi * B:(bi + 1) * B]
                    if T == 0:
                        nc.vector.memset(osl, 0.0)
                        continue
                    cv = c01_all[:, 2 * (off + boff):2 * (off + boff + T)]
                    tmp = ptmp.tile([128, B, 2 * t_max], F32, tag="tmp")
                    tv = tmp[:]
                    # tmp[p, b, u] = G[p, boff*64 + 32u + b] * c01[p, u]
                    gx = bass.AP(gv.tensor, gv.offset + boff * 2 * B,
                                 [list(gv.ap[0]), [B, 2 * T], [1, B]])
                    ab = bass.AP(cv.tensor, cv.offset,
                                 [list(cv.ap[0]), [1, 2 * T], [0, B]])
                    t_ap = bass.AP(tv.tensor, tv.offset,
                                   [list(tv.ap[0]), [1, 2 * T],
                                    [2 * t_max, B]])
                    nc.vector.tensor_mul(t_ap, gx, ab)

                    red_in = bass.AP(tv.tensor, tv.offset,
                                     [list(tv.ap[0]), [2 * t_max, B],
                                      [1, 2 * T]])
                    nc.vector.reduce_sum(osl, red_in,
                                         axis=mybir.AxisListType.X)
                    boff += T

            nc.sync.dma_start(out_d[:], ocore[:])

    nc.compile()
    return nc


def make_plan(x, forward_weights, forward_mask, output_mapping, reverse_mask):
    """Host-side planning: token extraction, core coloring, row pairing."""
    idx = np.asarray(output_mapping).astype(np.int64)
    rm = np.asarray(reverse_mask, dtype=np.float32)
    s = (np.asarray(forward_weights, dtype=np.float32)
         * np.asarray(forward_mask, dtype=np.float32))
    cols = np.arange(W)[None, :]
    c = rm * s[idx, cols]                                  # [O, W]
    nz = c != 0
    cnt = nz.sum(1)
    order = np.argsort(-cnt, kind="stable").tolist()

    # per-output token lists: (input, coeff) with duplicates aggregated
    toks = [None] * O
    for o in range(O):
        k = np.nonzero(nz[o])[0]
        ii = idx[o][k]
        cc = c[o][k]
        if len(ii) != len(set(ii.tolist())):
            agg = {}
            for i, cv in zip(ii.tolist(), cc.tolist()):
                agg[i] = agg.get(i, 0.0) + cv
            toks[o] = list(agg.items())
        else:
            toks[o] = list(zip(ii.tolist(), cc.tolist()))

    # ---- coloring: outputs sharing an input go to different cores
    input_mask = [0] * I
    core_load = [0] * NCORES
    cap = O // NCORES
    core_outputs = [[] for _ in range(NCORES)]
    for o in order:
        forb = 0
        for i, _ in toks[o]:
            forb |= input_mask[i]
        best, bestload = -1, 1 << 30
        for cc_ in range(NCORES):
            if core_load[cc_] >= cap or (forb >> cc_) & 1:
                continue
            if core_load[cc_] < bestload:
                best, bestload = cc_, core_load[cc_]
        if best < 0:
            bestkey = (1 << 30, 1 << 30)
            for cc_ in range(NCORES):
                if core_load[cc_] >= cap:
                    continue
                nconf = sum((input_mask[i] >> cc_) & 1 for i, _ in toks[o])
                key = (nconf, core_load[cc_])
                if key < bestkey:
                    bestkey, best = key, cc_
        core_outputs[best].append(o)
        core_load[best] += 1
        for i, _ in toks[o]:
            input_mask[i] |= 1 << best

    # ---- per-core greedy pairing into 256B rows
    plans = []
    for core in range(NCORES):
        placed = {}            # input -> (row, half)
        row_free = {}          # row -> free half
        nrows = 0
        slots = {}             # output -> list of [row, c0, c1]
        for o in core_outputs[core]:
            free = []
            touched = {}       # row -> [row, c0, c1]
            for i, cv in toks[o]:
                p = placed.get(i)
                if p is None:
                    free.append((i, cv))
                else:
                    sl = touched.get(p[0])
                    if sl is None:
                        sl = touched[p[0]] = [p[0], 0.0, 0.0]
                    sl[1 + p[1]] += cv
            nf = []
            for i, cv in free:
                done = False
                for r in touched:
                    h = row_free.pop(r, None)
                    if h is not None:
                        placed[i] = (r, h)
                        touched[r][1 + h] += cv
                        done = True
                        break
                if not done:
                    nf.append((i, cv))
            free = nf
            for g in range(len(free) // 2):
                (ia, ca), (ib, cb) = free[2 * g], free[2 * g + 1]
                if nrows >= NROWS:
                    raise RuntimeError("row overflow")
                placed[ia] = (nrows, 0)
                placed[ib] = (nrows, 1)
                touched[nrows] = [nrows, ca, cb]
                nrows += 1
            if len(free) % 2:
                i, cv = free[-1]
                r = None
                for rr in row_free:
                    if rr not in touched:
                        r = rr
                        break
                if r is not None:
                    h = row_free.pop(r)
                    placed[i] = (r, h)
                    sl = [r, 0.0, 0.0]
                    sl[1 + h] = cv
                    touched[r] = sl
                else:
                    if nrows >= NROWS:
                        raise RuntimeError("row overflow")
                    placed[i] = (nrows, 0)
                    row_free[nrows] = 1
                    touched[nrows] = [nrows, cv, 0.0]
                    nrows += 1
            slots[o] = list(touched.values())
        # row -> input map for the table
        row_inputs = np.zeros((NROWS, 2), np.int64)
        for i, (r, h) in placed.items():
            row_inputs[r, h] = i
        # sort outputs by slot count desc for uniform blocks
        ordered = sorted(core_outputs[core],
                         key=lambda o: -len(slots[o]))
        plans.append({"slots": slots, "ordered": ordered,
                      "row_inputs": row_inputs})

    # shared t_list across cores
    t_list = []
    for bi in range(NBLK):
        t = 0
        for pl in plans:
            blk = pl["ordered"][bi * 128:(bi + 1) * 128]
            t = max(t, max(len(pl["slots"][o]) for o in blk))
        t_list.append(t)
    t_list = tuple(t_list)

    # merge consecutive blocks into gathers of <= DESC_CAP descriptors
    groups = []
    a = 0
    while a < NBLK:
        b = a + 1
        tg = t_list[a]
        while b < NBLK and (tg + t_list[b]) * 128 <= DESC_CAP:
            tg += t_list[b]
            b += 1
        groups.append((a, b))
        a = b
    groups = tuple(groups)

    return {"plans": plans, "t_list": t_list, "groups": groups}


def make_in_maps(x, plan):
    x = np.asarray(x, dtype=np.float32)
    xT = np.ascontiguousarray(x.T)                         # [I, B]
    t_list = plan["t_list"]

    in_maps = []
    for core in range(NCORES):
        pl = plan["plans"][core]
        slots, ordered, row_inputs = (
            pl["slots"], pl["ordered"], pl["row_inputs"])
        tab = xT[row_inputs.reshape(-1)].reshape(NROWS, 2 * B)
        idx_parts, c_parts = [], []
        for bi, T in enumerate(t_list):
            outs = ordered[bi * 128:(bi + 1) * 128]
            if T == 0:
                continue
            J = np.zeros((128, T), np.int16)
            c01 = np.zeros((128, T, 2), np.float32)
            for p, o in enumerate(outs):
                for s_, (r, c0, c1) in enumerate(slots[o]):
                    J[p, s_] = r
                    c01[p, s_, 0] = c0
                    c01[p, s_, 1] = c1
            # token m = s*128 + p; wrap in 16 partitions, replicate x8
            L = J.T.reshape(8 * T, 16).T                   # [16, 8T]
            idx_parts.append(np.tile(L, (8, 1)))           # [128, 8T]
            c_parts.append(c01.reshape(128, 2 * T))
        in_maps.append({
            "tab": tab,
            "idx": np.ascontiguousarray(np.concatenate(idx_parts, axis=1)),
            "c01": np.ascontiguousarray(np.concatenate(c_parts, axis=1)),
        })
    return in_maps


def unshard_out(results, plan):
    out = np.empty((B, O), np.float32)
    for core in range(NCORES):
        oc = results[core]["out"]                          # [128, NBLK*B]
        vals = oc.reshape(128, NBLK, B).transpose(2, 1, 0) # [B, NBLK, 128]
        perm = np.array(plan["plans"][core]["ordered"])
        out[:, perm] = vals.reshape(B, NBLK * 128)
    return out


_NC = None
_NC_KEY = None


def _get_nc(t_list, groups):
    global _NC, _NC_KEY
    key = (t_list, groups)
    if _NC is None or _NC_KEY != key:
        _NC = _build_nc(t_list, groups)
        _NC_KEY = key
    return _NC


def kernel(x, forward_weights, forward_mask, output_mapping, reverse_mask):
    plan = make_plan(x, forward_weights, forward_mask,
                     output_mapping, reverse_mask)
    nc = _get_nc(plan["t_list"], plan["groups"])
    in_maps = make_in_maps(x, plan)
    res = run_bass_kernel_spmd(nc, in_maps, core_ids=list(range(NCORES)))
    return unshard_out(res.results, plan)
